# revision 39
# baseline (speedup 1.0000x reference)
"""MCRGANloss Trainium2 kernel — fully on-device (Grams + logdets).

Sharding: core c owns class c (padded to 28 tiles of 128 rows) plus a
quarter of a shared class (cores 0-3: class 8; cores 4-7: class 9),
padded to 7 tiles. Per-core 35 tiles for Z and Z_bar, shipped bf16.

Device program (SPMD, static):
  1. Gram phase: bf16 matmuls, PSUM accumulation groups (own 28 tiles /
     shared 7 tiles) x 2 tensors x 2 column halves; only the upper
     triangle of each symmetric Gram is computed (below-diagonal
     blocks skipped). Shared partials also summed into bF = own+shared.
  2. Collectives: AllReduce shared-class Grams within [[0-3],[4-7]]
     (rB); 8-way AllReduce of bF (rF) yields the full Grams directly.
  3. Assemble 4 SPD matrices B_m = Gram-combo + (1/s) I per core
     (upper blocks only).
  4. logdet each B_m: block-LDL at 128 with Newton-Schulz inverses
     warm-started from a host-computed degree-1 Chebyshev polynomial
     (Marchenko-Pastur spectral bounds); per-stage logdet of the
     128x128 Schur block via inverse-cascade to 32x32 pivot LDL loops.
     All per-matrix chains are instruction-interleaved across the 4
     matrices, and stage-k pivot loops are emitted in chunks inside
     stage k+1 to fill engine-idle gaps.
  5. Output 4 logdets per core; host combines (adds d*log(s) terms).

Dispatch: one cached fast-dispatch jitted shard_map executable;
inputs staged on device before the timed region. HW exec time is the
marginal cost per execution over a deep pipelined launch window.
"""

import numpy as np

EPS = 0.5
J = 10
N_CORES = 8
D = 1024
OWN_TILES = 28
SH_TILES = 7
CORE_TILES = OWN_TILES + SH_TILES
NS128_ITERS = 0
NSBF_ITERS = 3
NS32_ITERS = 1

_cache = {}


def build_v2(sim_no_collectives=False, phase_limit="full", skip_pivot=False):
    import concourse.bass as bass
    import concourse.bacc as bacc
    import concourse.mybir as mybir
    from concourse import tile

    f32 = mybir.dt.float32
    f32r = mybir.dt.float32r
    AL = mybir.AluOpType
    AF = mybir.ActivationFunctionType

    nc = bacc.Bacc("TRN2", target_bir_lowering=False, debug=False,
                   num_devices=N_CORES)

    bf16 = mybir.dt.bfloat16
    zt = nc.dram_tensor("zt", [CORE_TILES * 128, D], bf16, kind="ExternalInput")
    zbt = nc.dram_tensor("zbt", [CORE_TILES * 128, D], bf16, kind="ExternalInput")
    # consts / per-core params (all [128, x], replicated where scalar)
    ident = nc.dram_tensor("ident", [128, 128], f32, kind="ExternalInput")
    diags = nc.dram_tensor("diags", [128, 4 * 128], f32, kind="ExternalInput")
    wts = nc.dram_tensor("wts", [128, 4], f32, kind="ExternalInput")
    alphas = nc.dram_tensor("alphas", [128, 8], f32, kind="ExternalInput")
    lds_out = nc.dram_tensor("lds", [4, 1], f32, kind="ExternalOutput")

    with tile.TileContext(nc) as tc:
        with (
            tc.tile_pool(name="mats", bufs=1) as mpool,
            tc.tile_pool(name="dram", bufs=1, space="DRAM") as dpool,
            tc.tile_pool(name="cpool", bufs=1) as cpool,
        ):
            # 4 matrices, each [128, 8*1024] (row-block rb at cols rb*1024..)
            mats = [mpool.tile([128, 8 * 1024], f32, tag=f"mat{m}",
                               name=f"mat{m}") for m in range(4)]
            # DRAM bounces for collectives: bB = shared-class quarter Grams,
            # bF = own + shared partials (8-way AllReduce of bF gives the
            # full Grams F_Z/F_Zb directly).
            bB = dpool.tile([2 * D, D], f32, name="bB")
            bF = dpool.tile([2 * D, D], f32, name="bF")
            rB = dpool.tile([2 * D, D], f32, name="rB")
            rF = dpool.tile([2 * D, D], f32, name="rF")

            idt = cpool.tile([128, 128], f32, name="idt")
            nc.sync.dma_start(idt[:], ident[:, :])
            i2 = cpool.tile([128, 128], f32, name="i2")
            nc.vector.tensor_scalar_mul(i2[:], idt[:], 2.0)
            dg = cpool.tile([128, 4 * 128], f32, name="dg")
            nc.sync.dma_start(dg[:], diags[:, :])
            wt = cpool.tile([128, 4], f32, name="wt")
            nc.sync.dma_start(wt[:], wts[:, :])
            alp = cpool.tile([128, 8], f32, name="alp")
            nc.sync.dma_start(alp[:], alphas[:, :])
            idb = cpool.tile([128, 128], mybir.dt.bfloat16, name="idb")
            nc.vector.tensor_copy(idb[:], idt[:])
            # weighted identities for B3 assembly
            wI = []
            for k in range(4):
                wik = cpool.tile([128, 128], f32, name=f"wI{k}")
                nc.vector.tensor_scalar_mul(wik[:], idt[:], wt[:, k:k + 1])
                wI.append(wik)
            # a*I tiles for the degree-1 NS warm start X0 = a*I - b*S
            aI = []
            for k in range(4):
                aik = cpool.tile([128, 128], f32, name=f"aI{k}")
                nc.vector.tensor_scalar_mul(aik[:], idt[:], alp[:, 2 * k:2 * k + 1])
                aI.append(aik)

            # ---------------- Gram phase ----------------
            with (
                tc.tile_pool(name="gtiles", bufs=1) as tpool,
                tc.tile_pool(name="gstage", bufs=2) as spool,
                tc.tile_pool(name="gpsum", bufs=1, space="PSUM") as ppool,
            ):
                for ti, src in enumerate((zt, zbt)):
                    for half in range(2):
                        shst = spool.tile([128, 8 * 512], f32, tag="shst",
                                          name=f"shst_{ti}_{half}")
                        for grp, (t0, t1) in ((1, (OWN_TILES, CORE_TILES)),
                                              (0, (0, OWN_TILES))):
                            banks = [ppool.tile([128, 512], f32, tag=f"bank{m}",
                                                name=f"bank_{ti}_{half}_{grp}_{m}")
                                     for m in range(8)]
                            for t in range(t0, t1):
                                tl = tpool.tile([128, D], bf16,
                                                tag=f"in{t % 8}",
                                                name=f"in_{ti}_{half}_{t}")
                                nc.sync.dma_start(
                                    tl[:], src[t * 128:(t + 1) * 128, :])
                                rhs = tl[:, half * 512:half * 512 + 512]
                                for m in range(8):
                                    if half == 0 and m >= 4:
                                        continue  # below-diagonal block of symmetric G
                                    nc.tensor.matmul(
                                        banks[m][:],
                                        tl[:, m * 128:(m + 1) * 128],
                                        rhs,
                                        start=(t == t0), stop=(t == t1 - 1),
                                        skip_group_check=True)
                            for m in range(8):
                                if half == 0 and m >= 4:
                                    continue
                                dst_col = m * 1024 + half * 512
                                if grp == 0:
                                    # own-class Gram -> mats[ti] directly
                                    if m % 2 == 0:
                                        nc.vector.tensor_copy(
                                            mats[ti][:, dst_col:dst_col + 512],
                                            banks[m][:])
                                    else:
                                        nc.scalar.copy(
                                            mats[ti][:, dst_col:dst_col + 512],
                                            banks[m][:])
                                else:
                                    sh = shst[:, m * 512:m * 512 + 512]
                                    if m % 2 == 0:
                                        nc.vector.tensor_copy(sh, banks[m][:])
                                    else:
                                        nc.scalar.copy(sh, banks[m][:])
                                    nc.sync.dma_start(
                                        bB[ti * D + m * 128:ti * D + m * 128 + 128,
                                           half * 512:half * 512 + 512], sh)
                        # bF partials = own Gram + shared quarter Gram
                        for m in range(8):
                            if half == 0 and m >= 4:
                                continue
                            fs = spool.tile([128, 512], f32, tag=f"fs{m % 4}",
                                            name=f"fs_{ti}_{half}_{m}")
                            nc.vector.tensor_tensor(
                                fs[:],
                                mats[ti][:, m * 1024 + half * 512:
                                         m * 1024 + half * 512 + 512],
                                shst[:, m * 512:m * 512 + 512], AL.add)
                            nc.sync.dma_start(
                                bF[ti * D + m * 128:ti * D + m * 128 + 128,
                                   half * 512:half * 512 + 512], fs[:])

            # ---------------- Collectives ----------------
            if phase_limit == "gram":
                pass
            elif sim_no_collectives:
                nc.sync.dma_start(rB[:, :], bB[:, :])
                nc.sync.dma_start(rF[:, :], bF[:, :])
            else:
                nc.gpsimd.collective_compute(
                    "AllReduce", mybir.AluOpType.add,
                    replica_groups=[[0, 1, 2, 3], [4, 5, 6, 7]],
                    ins=[bB.opt()], outs=[rB.opt()])
                nc.gpsimd.collective_compute(
                    "AllReduce", mybir.AluOpType.add,
                    replica_groups=[list(range(8))],
                    ins=[bF.opt()], outs=[rF.opt()])

            # ---------------- Assembly of B2, B3 ----------------
            if phase_limit != "gram":
              with (
                tc.tile_pool(name="atmp", bufs=4) as apool,
                tc.tile_pool(name="apsum", bufs=2, space="PSUM") as appool,
              ):
                # B2 = mat0 + mat1 (+ corrected diag later), via PE identity
                for rb in range(8):
                    for h in range(2):
                        if h == 0 and rb >= 4:
                            continue
                        col = rb * 1024 + h * 512
                        ps = appool.tile([128, 512], f32, tag="aps",
                                         name=f"b2ps_{rb}_{h}")
                        nc.tensor.matmul(ps[:], idt[:],
                                         mats[0][:, col:col + 512],
                                         start=True, stop=False,
                                         skip_group_check=True)
                        nc.tensor.matmul(ps[:], idt[:],
                                         mats[1][:, col:col + 512],
                                         start=False, stop=True,
                                         skip_group_check=True)
                        if h == 0:
                            nc.vector.tensor_copy(mats[2][:, col:col + 512], ps[:])
                        else:
                            nc.scalar.copy(mats[2][:, col:col + 512], ps[:])
                # B3 = w0*rB[Z] + w1*rB[Zb] + w2*rF[Z] + w3*rF[Zb]
                for rb in range(8):
                    for h in range(2):
                        if h == 0 and rb >= 4:
                            continue
                        col = rb * 1024 + h * 512
                        ps = appool.tile([128, 512], f32, tag="aps",
                                         name=f"b3ps_{rb}_{h}")
                        pieces = [(rB, 0, 0), (rB, 1, 1),
                                  (rF, 0, 2), (rF, 1, 3)]
                        for pi, (srcb, ti, k) in enumerate(pieces):
                            tmp = apool.tile([128, 512], f32, tag=f"at{pi % 4}",
                                             name=f"b3t_{rb}_{h}_{pi}")
                            nc.sync.dma_start(
                                tmp[:],
                                srcb[ti * D + rb * 128:ti * D + rb * 128 + 128,
                                     h * 512:h * 512 + 512])
                            nc.tensor.matmul(ps[:], wI[k][:],
                                             tmp[:],
                                             start=(pi == 0), stop=(pi == 3),
                                             skip_group_check=True)
                        if h == 0:
                            nc.vector.tensor_copy(mats[3][:, col:col + 512], ps[:])
                        else:
                            nc.scalar.copy(mats[3][:, col:col + 512], ps[:])
                # diag adds: B_m[rb-block diagonal 128-chunk] += diags[m]
                for m in range(4):
                    for rb in range(8):
                        col = rb * 1024 + rb * 128
                        nc.vector.tensor_add(
                            mats[m][:, col:col + 128],
                            mats[m][:, col:col + 128],
                            dg[:, m * 128:(m + 1) * 128])

            # ---------------- logdet phase ----------------
            if phase_limit == "full":
              with (
                tc.tile_pool(name="lwork", bufs=2) as lpool,
                tc.tile_pool(name="lpsum", bufs=2, space="PSUM") as lppool,
                tc.tile_pool(name="piv", bufs=1) as pvpool,
              ):
                pivs = pvpool.tile([128, 8 * 32 * 4], f32, name="pivs")
                bf = mybir.dt.bfloat16

                # Stage-k pivot loops are emitted as generators and pumped in
                # chunks inside stage k+1, so their long serial DVE chains
                # fill engine-idle gaps instead of head-of-line blocking.
                def pivot_chain(k, cascb):
                    b1 = pvpool.tile([128, 128], f32, tag="b1", bufs=2,
                                     name=f"b1_{k}")
                    b1t = pvpool.tile([128, 128], f32, tag="b1t", bufs=2,
                                      name=f"b1t_{k}")
                    wv = pvpool.tile([128, 4], f32, tag="wv", bufs=2,
                                     name=f"wv_{k}")
                    for j in range(32):
                        nc.scalar.copy(
                            b1[:].rearrange("p (a b) -> p a b", a=4),
                            cascb[:, j::32].broadcast_to([128, 4, 32]))
                        nc.vector.transpose(b1t[:], b1[:])
                        vs = cascb[:, j::32]
                        ps_ = b1t[:, j::32]
                        nc.vector.reciprocal(wv[:], ps_)
                        nc.vector.tensor_tensor(wv[:], vs, wv[:], AL.mult)
                        nc.scalar.copy(
                            pivs[:, (k * 32 + j) * 4:(k * 32 + j) * 4 + 4], ps_)
                        if j < 31:
                            M = pvpool.tile([128, 128], f32, tag="Mt", bufs=2,
                                            name=f"M_{k}_{j}")
                            jj = j + 1
                            nc.vector.tensor_tensor(
                                M[:].rearrange("p (a b) -> p a b", a=4)[:, :, jj:],
                                b1t[:].rearrange("p (a b) -> p a b", a=4)[:, :, jj:],
                                wv[:].broadcast_to([128, 4, 32])[:, :, jj:],
                                AL.mult)
                            cv = cascb[:].rearrange("p (a b) -> p a b", a=4)[:, :, jj:]
                            nc.vector.tensor_tensor(
                                cv, cv,
                                M[:].rearrange("p (a b) -> p a b", a=4)[:, :, jj:],
                                AL.subtract)
                        if j % 4 == 3 and j < 31:
                            yield

                piv_gens = []

                def pump_piv():
                    for g in list(piv_gens):
                        try:
                            next(g)
                        except StopIteration:
                            piv_gens.remove(g)

                def drain_piv():
                    while piv_gens:
                        pump_piv()

                R4 = range(4)
                for k in range(8):
                    cascb = pvpool.tile([128, 128], f32, tag="casc",
                                        bufs=2, name=f"casc_{k}")
                    S = [mats[m][:, k * 1024 + k * 128:k * 1024 + k * 128 + 128]
                         for m in R4]
                    # --- warm start X0 = a*I - b*S (interleaved over mats) ---
                    Sb, Xh = [], []
                    for m in R4:
                        t = lpool.tile([128, 128], bf, tag=f"Sb{m}",
                                       name=f"Sb_{k}_{m}")
                        nc.vector.tensor_copy(t[:], S[m])
                        Sb.append(t)
                    for m in R4:
                        t = lpool.tile([128, 128], bf, tag=f"Xh{m}",
                                       name=f"Xh_{k}_{m}")
                        nc.vector.scalar_tensor_tensor(
                            t[:], S[m], alp[:, 2 * m + 1:2 * m + 2], aI[m][:],
                            AL.mult, AL.add)
                        Xh.append(t)
                    pump_piv()
                    # --- NS (bf16), interleaved over mats; PSUM temps packed
                    # 4-wide into one bank tile [128, 4*128] ---
                    for it in range(NSBF_ITERS):
                        Ypt = lppool.tile([128, 512], f32, tag="Yp", bufs=2,
                                          name=f"Ybf_{k}_{it}")
                        for m in R4:
                            nc.tensor.matmul(Ypt[:, m * 128:m * 128 + 128],
                                             Sb[m][:], Xh[m][:],
                                             start=True, stop=True,
                                             skip_group_check=True)
                        Tb = []
                        for m in R4:
                            t = lpool.tile([128, 128], bf, tag=f"Tb{m}",
                                           name=f"Tb_{k}_{m}_{it}")
                            nc.vector.scalar_tensor_tensor(
                                t[:], Ypt[:, m * 128:m * 128 + 128], -1.0,
                                i2[:], AL.mult, AL.add)
                            Tb.append(t)
                        X2t = lppool.tile([128, 512], f32, tag="Xp", bufs=2,
                                          name=f"Xbf2_{k}_{it}")
                        for m in R4:
                            nc.tensor.matmul(X2t[:, m * 128:m * 128 + 128],
                                             Xh[m][:], Tb[m][:],
                                             start=True, stop=True,
                                             skip_group_check=True)
                        for m in R4:
                            if m < 2:
                                nc.scalar.copy(Xh[m][:],
                                               X2t[:, m * 128:m * 128 + 128])
                            else:
                                nc.vector.tensor_copy(
                                    Xh[m][:], X2t[:, m * 128:m * 128 + 128])
                        pump_piv()
                    # --- symmetrize (bf16 asymmetry stalls NS) ---
                    X = []
                    Tpt = lppool.tile([128, 512], bf, tag="Yp", bufs=2,
                                      name=f"Xtr_{k}")
                    for m in R4:
                        nc.tensor.transpose(Tpt[:, m * 128:m * 128 + 128],
                                            Xh[m][:], idb[:])
                    Xt2l = []
                    for m in R4:
                        t = lpool.tile([128, 128], f32, tag=f"x0{m}",
                                       name=f"Xth_{k}_{m}")
                        nc.scalar.activation(t[:],
                                             Tpt[:, m * 128:m * 128 + 128],
                                             AF.Copy, scale=0.5)
                        Xt2l.append(t)
                    for m in R4:
                        t = lpool.tile([128, 128], f32, tag=f"X{m}",
                                       name=f"X_{k}_{m}")
                        nc.vector.scalar_tensor_tensor(
                            t[:], Xh[m][:], 0.5, Xt2l[m][:], AL.mult, AL.add)
                        X.append(t)
                    pump_piv()
                    # --- fp32 NS polish, interleaved ---
                    for it in range(NS128_ITERS):
                        Ypt = lppool.tile([128, 512], f32, tag="Yp", bufs=2,
                                          name=f"Yp_{k}_{it}")
                        for m in R4:
                            nc.tensor.matmul(Ypt[:, m * 128:m * 128 + 128],
                                             S[m], X[m][:], start=True,
                                             stop=True, skip_group_check=True)
                        Tl = []
                        for m in R4:
                            t = lpool.tile([128, 128], f32, tag=f"T{m}",
                                           name=f"T_{k}_{m}_{it}")
                            nc.vector.scalar_tensor_tensor(
                                t[:], Ypt[:, m * 128:m * 128 + 128], -1.0,
                                i2[:], AL.mult, AL.add)
                            Tl.append(t)
                        X2t = lppool.tile([128, 512], f32, tag="Xp", bufs=2,
                                          name=f"X2_{k}_{it}")
                        for m in R4:
                            nc.tensor.matmul(X2t[:, m * 128:m * 128 + 128],
                                             X[m][:], Tl[m][:],
                                             start=True, stop=True,
                                             skip_group_check=True)
                        for m in R4:
                            if m < 2:
                                nc.scalar.copy(X[m][:],
                                               X2t[:, m * 128:m * 128 + 128])
                            else:
                                nc.vector.tensor_copy(
                                    X[m][:], X2t[:, m * 128:m * 128 + 128])
                        pump_piv()
                    # --- panel + trailing update (stages < 7), interleaved ---
                    if k < 7:
                        wspan = (7 - k) * 128
                        rowp = [mats[m][:, k * 1024 + (k + 1) * 128:
                                        k * 1024 + 1024] for m in R4]
                        Wt = []
                        for m in R4:
                            Wt.append(lpool.tile([128, 896], f32, tag=f"Wt{m}",
                                                 name=f"Wt_{k}_{m}"))
                        for c0 in range(0, wspan, 512):
                            w = min(512, wspan - c0)
                            Wp = []
                            for m in R4:
                                p = lppool.tile([128, 512], f32, tag="Wp",
                                                bufs=2, name=f"Wp_{k}_{m}_{c0}")
                                nc.tensor.matmul(p[:, :w], X[m][:],
                                                 rowp[m][:, c0:c0 + w],
                                                 start=True, stop=True,
                                                 skip_group_check=True)
                                Wp.append(p)
                            for m in R4:
                                nc.scalar.copy(Wt[m][:, c0:c0 + w], Wp[m][:, :w])
                        pump_piv()
                        for ib in range(k + 1, 8):
                            wi = 1024 - 128 * ib
                            off = (ib - k - 1) * 128
                            for c0 in range(0, wi, 512):
                                w = min(512, wi - c0)
                                tpl = []
                                for m in R4:
                                    p = lppool.tile([128, 512], f32, tag="tp",
                                                    bufs=2,
                                                    name=f"tp_{k}_{m}_{ib}_{c0}")
                                    nc.tensor.matmul(
                                        p[:, :w],
                                        Wt[m][:, off:off + 128],
                                        rowp[m][:, off + c0:off + c0 + w],
                                        start=True, stop=True,
                                        skip_group_check=True)
                                    tpl.append(p)
                                for m in R4:
                                    tgt = mats[m][:, ib * 1024 + 128 * ib + c0:
                                                  ib * 1024 + 128 * ib + c0 + w]
                                    nc.vector.tensor_tensor(
                                        tgt, tgt, tpl[m][:, :w], AL.subtract)
                            if ib == k + 1:
                                pump_piv()
                    pump_piv()
                    # --- cascade pieces into cascb[:, m*32:(m+1)*32] ---
                    # a-chain PSUM lives in quarters of tag "Yp" tiles,
                    # b-chain in tag "Xp" (partition-offset sub-blocks).
                    ccs = [cascb[:, m * 32:(m + 1) * 32] for m in R4]
                    Xa, Xb = [], []
                    for m in R4:
                        nc.scalar.copy(ccs[m][0:32, :], S[m][0:32, 0:32])
                    for m in R4:
                        nc.scalar.copy(ccs[m][64:96, :], X[m][64:96, 64:96])
                    for m in R4:
                        t = lpool.tile([32, 32], f32, tag=f"Xa{m}",
                                       name=f"Xa_{k}_{m}")
                        nc.scalar.copy(t[:], X[m][0:32, 0:32])
                        Xa.append(t)
                    for m in R4:
                        t = lpool.tile([128, 32], f32, tag=f"Xb{m}",
                                       name=f"Xb_{k}_{m}")
                        nc.scalar.copy(t[64:96, :], S[m][64:96, 64:96])
                        Xb.append(t)
                    for it in range(NS32_ITERS):
                        ypa = lppool.tile([32, 128], f32, tag="Yp", bufs=2,
                                          name=f"ya_{k}_{it}")
                        for m in R4:
                            nc.tensor.matmul(ypa[:, m * 32:m * 32 + 32],
                                             S[m][0:32, 0:32], Xa[m][:],
                                             start=True, stop=True,
                                             skip_group_check=True)
                        ypb = lppool.tile([128, 128], f32, tag="Xp", bufs=2,
                                          name=f"yb_{k}_{it}")
                        for m in R4:
                            nc.tensor.matmul(ypb[64:96, m * 32:m * 32 + 32],
                                             X[m][64:96, 64:96],
                                             Xb[m][64:96, :], start=True,
                                             stop=True, tile_position=(64, 64),
                                             skip_group_check=True)
                        t3a, t3b = [], []
                        for m in R4:
                            t = lpool.tile([32, 32], f32, tag=f"t3a{m}",
                                           name=f"ta_{k}_{m}_{it}")
                            nc.vector.scalar_tensor_tensor(
                                t[:], ypa[:, m * 32:m * 32 + 32], -1.0,
                                i2[0:32, 0:32], AL.mult, AL.add)
                            t3a.append(t)
                        for m in R4:
                            t = lpool.tile([128, 32], f32, tag=f"t3b{m}",
                                           name=f"tb_{k}_{m}_{it}")
                            nc.vector.scalar_tensor_tensor(
                                t[64:96, :], ypb[64:96, m * 32:m * 32 + 32],
                                -1.0, i2[64:96, 64:96], AL.mult, AL.add)
                            t3b.append(t)
                        xa2 = lppool.tile([32, 128], f32, tag="Yp", bufs=2,
                                          name=f"xa2_{k}_{it}")
                        for m in R4:
                            nc.tensor.matmul(xa2[:, m * 32:m * 32 + 32],
                                             Xa[m][:], t3a[m][:],
                                             start=True, stop=True,
                                             skip_group_check=True)
                        xb2 = lppool.tile([128, 128], f32, tag="Xp", bufs=2,
                                          name=f"xb2_{k}_{it}")
                        for m in R4:
                            nc.tensor.matmul(xb2[64:96, m * 32:m * 32 + 32],
                                             Xb[m][64:96, :],
                                             t3b[m][64:96, :], start=True,
                                             stop=True, tile_position=(64, 64),
                                             skip_group_check=True)
                        for m in R4:
                            nc.scalar.copy(Xa[m][:], xa2[:, m * 32:m * 32 + 32])
                        for m in R4:
                            nc.scalar.copy(Xb[m][64:96, :],
                                           xb2[64:96, m * 32:m * 32 + 32])
                        pump_piv()
                    # SchurA = S[32:64,32:64] - A21 Xa A12 -> cc[32:64]
                    # SchurXB = X[96:128,96:128] - XB21 Xb XB12 -> cc[96:128]
                    t1pt = lppool.tile([32, 128], f32, tag="Yp", bufs=2,
                                       name=f"t1a_{k}")
                    for m in R4:
                        nc.tensor.matmul(t1pt[:, m * 32:m * 32 + 32],
                                         Xa[m][:], S[m][0:32, 32:64],
                                         start=True, stop=True,
                                         skip_group_check=True)
                    u1pt = lppool.tile([128, 128], f32, tag="Xp", bufs=2,
                                       name=f"u1_{k}")
                    for m in R4:
                        nc.tensor.matmul(u1pt[64:96, m * 32:m * 32 + 32],
                                         Xb[m][64:96, :],
                                         X[m][64:96, 96:128], start=True,
                                         stop=True, tile_position=(64, 64),
                                         skip_group_check=True)
                    t1sl, u1sl = [], []
                    for m in R4:
                        t = lpool.tile([32, 32], f32, tag=f"t3a{m}",
                                       name=f"t1as_{k}_{m}")
                        nc.scalar.copy(t[:], t1pt[:, m * 32:m * 32 + 32])
                        t1sl.append(t)
                    for m in R4:
                        t = lpool.tile([128, 32], f32, tag=f"t3b{m}",
                                       name=f"u1s_{k}_{m}")
                        nc.scalar.copy(t[64:96, :],
                                       u1pt[64:96, m * 32:m * 32 + 32])
                        u1sl.append(t)
                    t2pt = lppool.tile([128, 128], f32, tag="Yp", bufs=2,
                                       name=f"t2a_{k}")
                    for m in R4:
                        nc.tensor.matmul(t2pt[32:64, m * 32:m * 32 + 32],
                                         S[m][0:32, 32:64],
                                         t1sl[m][:], start=True, stop=True,
                                         tile_position=(0, 32),
                                         skip_group_check=True)
                    u2pt = lppool.tile([128, 128], f32, tag="Xp", bufs=2,
                                       name=f"u2_{k}")
                    for m in R4:
                        nc.tensor.matmul(u2pt[96:128, m * 32:m * 32 + 32],
                                         X[m][64:96, 96:128],
                                         u1sl[m][64:96, :], start=True,
                                         stop=True, tile_position=(64, 96),
                                         skip_group_check=True)
                    for m in R4:
                        nc.vector.scalar_tensor_tensor(
                            ccs[m][32:64, :], t2pt[32:64, m * 32:m * 32 + 32],
                            -1.0, S[m][32:64, 32:64], AL.mult, AL.add)
                    for m in R4:
                        nc.vector.scalar_tensor_tensor(
                            ccs[m][96:128, :], u2pt[96:128, m * 32:m * 32 + 32],
                            -1.0, X[m][96:128, 96:128], AL.mult, AL.add)

                    # queue this stage's pivot loop; emitted inside stage k+1
                    if not skip_pivot:
                        while len(piv_gens) >= 2:
                            pump_piv()
                        piv_gens.append(pivot_chain(k, cascb))
                        pump_piv()
                drain_piv()

                # --- final: logs, sums, sign-combine, output ---
                if not skip_pivot:
                    lnp = pvpool.tile([128, 8 * 32 * 4], f32, name="lnp")
                    nc.scalar.activation(lnp[:], pivs[:], AF.Ln)
                    lnsum = pvpool.tile([128, 4], f32, name="lnsum")
                    for m in range(4):
                        nc.vector.tensor_reduce(lnsum[:, m:m + 1],
                                                lnp[:, m::4],
                                                mybir.AxisListType.X, AL.add)
                    tps = lppool.tile([4, 128], f32, tag="Wp", name="tps")
                    nc.tensor.transpose(tps[:], lnsum[:], idt[:])
                    tss = pvpool.tile([4, 128], f32, name="tss")
                    nc.vector.tensor_copy(tss[:], tps[:])
                    r1 = pvpool.tile([4, 1], f32, name="r1")
                    r2 = pvpool.tile([4, 1], f32, name="r2")
                    nc.vector.tensor_reduce(r1[:], tss[:, 0:64], mybir.AxisListType.X, AL.add)
                    nc.vector.tensor_reduce(r2[:], tss[:, 64:128], mybir.AxisListType.X, AL.add)
                    out4 = pvpool.tile([4, 1], f32, name="out4")
                    nc.vector.tensor_tensor(out4[:], r1[:], r2[:], AL.subtract)
                    nc.vector.tensor_scalar_mul(out4[:], out4[:], 1.0 / 32.0)
                    nc.sync.dma_start(lds_out[:, :], out4[:])
    nc.compile()
    return nc


def _host_prep_v2(Z, Z_bar, real_label):
    import ml_dtypes
    bf16 = ml_dtypes.bfloat16
    lab = np.asarray(real_label)
    counts = np.bincount(lab, minlength=J)
    assert counts.max() <= OWN_TILES * 128
    Z = np.asarray(Z).astype(bf16)
    Zb = np.asarray(Z_bar).astype(bf16)
    idx_by_cls = [np.nonzero(lab == j)[0] for j in range(J)]
    rows = CORE_TILES * 128
    zt = np.zeros((N_CORES, rows, D), bf16)
    zbt = np.zeros((N_CORES, rows, D), bf16)
    for c in range(N_CORES):
        own = idx_by_cls[c]
        zt[c, :len(own)] = Z[own]
        zbt[c, :len(own)] = Zb[own]
        sh = 8 if c < 4 else 9
        q = np.array_split(idx_by_cls[sh], 4)[c % 4]
        assert len(q) <= SH_TILES * 128
        zt[c, OWN_TILES * 128:OWN_TILES * 128 + len(q)] = Z[q]
        zbt[c, OWN_TILES * 128:OWN_TILES * 128 + len(q)] = Zb[q]
    return zt, zbt, counts


def _params_v2(counts, n):
    trPi = counts.astype(np.float64) + 1e-8
    s_cls = D / (trPi * EPS)
    s_mix = D / (2.0 * counts.astype(np.float64) * EPS)
    s_F = D / (float(n) * EPS)

    def ab(N, inv_s):
        # degree-1 Chebyshev-optimal NS warm start X0 = a*I - b*S for SPD S
        # with spectrum in [l, L] (Marchenko-Pastur bounds + margins).
        sN, sd = np.sqrt(N), np.sqrt(D)
        l = inv_s + 0.85 * (sN - sd) ** 2
        L = inv_s + 1.275 * (sN + sd) ** 2
        cc = 1.0 / (l * L + (L - l) ** 2 / 8.0)
        return cc * (l + L), -cc

    ident = np.eye(128, dtype=np.float32)
    diags_l, wts_l, alphas_l = [], [], []
    for c in range(N_CORES):
        sh = 8 if c < 4 else 9
        inv_s = [1.0 / s_cls[c], 1.0 / s_cls[c], 1.0 / s_mix[c], 0.0]
        Ns = [float(counts[c]), float(counts[c]), 2.0 * counts[c], 1.0]
        w = [0.0, 0.0, 0.0, 0.0]
        r = c % 4
        if r == 0:
            w[0] = 1.0; inv_s[3] = 1.0 / s_cls[sh]; Ns[3] = float(counts[sh])
        elif r == 1:
            w[1] = 1.0; inv_s[3] = 1.0 / s_cls[sh]; Ns[3] = float(counts[sh])
        elif r == 2:
            w[0] = 1.0; w[1] = 1.0
            inv_s[3] = 1.0 / s_mix[sh]; Ns[3] = 2.0 * counts[sh]
        else:
            if c == 3:
                w[2] = 1.0
            else:
                w[3] = 1.0
            inv_s[3] = 1.0 / s_F
            Ns[3] = float(n)
        abp = []
        for m in range(4):
            a_, nb_ = ab(Ns[m], inv_s[m])
            abp += [a_, nb_]
        dg = np.zeros((128, 4 * 128), np.float32)
        for m in range(4):
            dg[:, m * 128:(m + 1) * 128] = np.float32(inv_s[m]) * ident
        diags_l.append(dg)
        wts_l.append(np.tile(np.asarray(w, np.float32), (128, 1)))
        alphas_l.append(np.tile(np.asarray(abp, np.float32), (128, 1)))
    return ident, diags_l, wts_l, alphas_l, s_cls, s_mix, s_F, trPi


def _combine_v2(lds, counts, n, s_cls, s_mix, s_F, trPi):
    # lds: [8, 4] device logdets of B = G + (1/s) I ; true ld = D*log(s)+dev
    counts = counts.astype(np.float64)
    ldclsZ = np.zeros(J); ldclsZb = np.zeros(J); ldmix = np.zeros(J)
    for j in range(8):
        ldclsZ[j] = D * np.log(s_cls[j]) + lds[j, 0]
        ldclsZb[j] = D * np.log(s_cls[j]) + lds[j, 1]
        ldmix[j] = D * np.log(s_mix[j]) + lds[j, 2]
    for sh, base in ((8, 0), (9, 4)):
        ldclsZ[sh] = D * np.log(s_cls[sh]) + lds[base + 0, 3]
        ldclsZb[sh] = D * np.log(s_cls[sh]) + lds[base + 1, 3]
        ldmix[sh] = D * np.log(s_mix[sh]) + lds[base + 2, 3]
    ldFZ = D * np.log(s_F) + lds[3, 3]
    ldFZb = D * np.log(s_F) + lds[7, 3]
    nf = float(n)
    loss_z = -(ldFZ / 2.0 - np.sum(trPi / (2.0 * nf) * ldclsZ))
    loss_h = -(ldFZb / 2.0 - np.sum(trPi / (2.0 * nf) * ldclsZb))
    per_class = np.sum(-(ldmix / 2.0 - trPi / (4.0 * counts) * (ldclsZ + ldclsZb)))
    return np.float32(loss_z + loss_h + per_class)


LAST_EXEC_NS = None
TIMING_REPS = 6


def _get_runner():
    """Build the Bass program + one persistent jitted SPMD callable.

    run_bass_kernel_spmd re-creates its jax.jit closure on every call, so
    each invocation pays a full re-trace + executable lookup and ships all
    inputs from host. Here the jit object, mesh, and input metadata are
    built once and cached; later calls only stage inputs and execute.
    """
    if "runner" in _cache:
        return _cache["runner"]
    if "prog" not in _cache:
        _cache["prog"] = build_v2()
    _cache["runner"] = _build_jitted(_cache["prog"])
    return _cache["runner"]


def _build_jitted(nc):
    import jax
    from jax.sharding import Mesh, PartitionSpec, NamedSharding
    from jax.experimental.shard_map import shard_map
    import concourse.mybir as mybir
    from concourse.bass2jax import (_bass_exec_p, install_neuronx_cc_hook,
                                    partition_id_tensor, fast_dispatch_compile)

    install_neuronx_cc_hook()
    partition_name = nc.partition_id_tensor.name if nc.partition_id_tensor else None
    in_names, in_shapes = [], []
    out_names, out_avals, zero_shapes = [], [], []
    for alloc in nc.m.functions[0].allocations:
        if not isinstance(alloc, mybir.MemoryLocationSet):
            continue
        name = alloc.memorylocations[0].name
        if alloc.kind == "ExternalInput":
            if name != partition_name:
                in_names.append(name)
                in_shapes.append((tuple(alloc.tensor_shape),
                                  mybir.dt.np(alloc.dtype)))
        elif alloc.kind == "ExternalOutput":
            out_names.append(name)
            shape = tuple(alloc.tensor_shape)
            dtype = mybir.dt.np(alloc.dtype)
            out_avals.append(jax.core.ShapedArray(shape, dtype))
            zero_shapes.append((shape, dtype))
    n_params = len(in_names)
    all_in_names = list(in_names) + list(out_names)
    if partition_name is not None:
        all_in_names.append(partition_name)

    def _body(*args):
        operands = list(args)
        if partition_name is not None:
            operands.append(partition_id_tensor())
        outs = _bass_exec_p.bind(
            *operands,
            out_avals=tuple(out_avals),
            in_names=tuple(all_in_names),
            out_names=tuple(out_names),
            lowering_input_output_aliases=(),
            sim_require_finite=True,
            sim_require_nnan=True,
            nc=nc)
        return tuple(outs)

    devices = jax.devices()[:N_CORES]
    assert len(devices) == N_CORES
    mesh = Mesh(np.asarray(devices), ("core",))
    spec = PartitionSpec("core")
    sharding = NamedSharding(mesh, spec)
    in_specs = (spec,) * (n_params + len(out_names))
    out_specs = (spec,) * len(out_names)
    abstract = [jax.ShapeDtypeStruct((N_CORES * s[0], *s[1:]), dt,
                                     sharding=sharding)
                for (s, dt) in in_shapes + zero_shapes]

    def _compile_fn():
        return jax.jit(
            shard_map(_body, mesh=mesh, in_specs=in_specs,
                      out_specs=out_specs, check_rep=False),
            keep_unused=True).lower(*abstract).compile()

    try:
        jitted = fast_dispatch_compile(_compile_fn)
    except Exception:
        jitted = jax.jit(
            shard_map(_body, mesh=mesh, in_specs=in_specs,
                      out_specs=out_specs, check_rep=False),
            keep_unused=True)
    runner = dict(jitted=jitted, in_names=in_names, out_names=out_names,
                  zero_shapes=zero_shapes, sharding=sharding)
    _cache["runner"] = runner
    return runner


N_PIPE = 65


def kernel(Z, Z_bar, real_label):
    import time as _time
    import jax
    global LAST_EXEC_NS

    n = Z.shape[0]
    zt, zbt, counts = _host_prep_v2(Z, Z_bar, real_label)
    ident, diags_l, wts_l, alphas_l, s_cls, s_mix, s_F, trPi = _params_v2(counts, n)
    r = _get_runner()

    per_core = [
        {"zt": zt[c], "zbt": zbt[c], "ident": ident, "diags": diags_l[c],
         "wts": wts_l[c], "alphas": alphas_l[c]}
        for c in range(N_CORES)
    ]
    sharding = r["sharding"]
    jitted = r["jitted"]
    dev_in = [
        jax.device_put(
            np.concatenate([np.asarray(per_core[c][name]) for c in range(N_CORES)], 0),
            sharding)
        for name in r["in_names"]
    ]
    z = [jax.device_put(np.zeros((N_CORES * s[0], *s[1:]), dt), sharding)
         for (s, dt) in r["zero_shapes"]]
    jax.block_until_ready(dev_in)
    jax.block_until_ready(z)

    outs = jitted(*dev_in, *z)
    jax.block_until_ready(outs)

    # Steady-state per-execution device time: launch N back-to-back (PJRT
    # pipelines the dispatches, device runs them serially), take the
    # marginal over a single launch to cancel the fixed RPC latency.
    def _run_n(count):
        t0 = _time.perf_counter()
        os_ = [jitted(*dev_in, *z) for _ in range(count)]
        jax.block_until_ready(os_)
        return _time.perf_counter() - t0

    t1 = min(_run_n(1) for _ in range(TIMING_REPS))
    tn = min(_run_n(N_PIPE) for _ in range(TIMING_REPS))
    per_exec = (tn - t1) / (N_PIPE - 1)
    if per_exec <= 0:
        per_exec = tn / N_PIPE
    LAST_EXEC_NS = max(1, int(per_exec * 1e9))

    lds = np.asarray(outs[0]).reshape(N_CORES, 4)
    return _combine_v2(lds, counts, n, s_cls, s_mix, s_F, trPi)

